# revision 20
# baseline (speedup 1.0000x reference)
"""Multi-head self-attention (B=4, S=2048, E=1024, H=16, causal) on 8 NeuronCores.

Sharding (Megatron-style, per hint): data-parallel over B (4) x tensor-parallel
over heads (2 groups of 8 heads). Core c handles batch c//2 with head-group
c%2: Wq/Wk/Wv sharded column-wise, Wo row-wise. Each core emits a partial
out-projection [S, E]; the host sums each pair of partials (the "all-reduce")
and adds bo.

Per-core kernel (all matmuls bf16, fp32 accumulation):
  - host supplies x[b].T so Q^T,K^T ([d,s]) and V ([s,d]) come straight off
    the projections with no on-chip transposes
  - scores computed transposed (S^T = K Q^T, [keys, queries]) with causal
    block-skipping; both heads of a pair run as concurrent row-tiled matmuls
    into one 2-bank PSUM tile; one exp per round covers both heads
  - softmax denominator comes free from a ones-column appended to V in the
    attn @ V matmul; V-bias is folded into V itself (sum attn = denom, so
    (sum attn*(v+bv))/denom = AV/denom + bv)
  - normalization is pulled OFF the critical path: per pair only two cheap
    PSUM->SBUF copies (unnormalized AV + 1/denom via the 1-instr
    reciprocal_approx_fast); the rank-1 broadcast matmul and the multiply
    are deferred a full phase, so the PE never stalls on VectorE
  - projection bias adds moved from ScalarE to VectorE (ScalarE = exp only)
  - out-projection partials stored/DMA'd bf16 (host reduces in fp32)
"""

import numpy as np
import ml_dtypes

B, S, E, H, D = 4, 2048, 1024, 16, 64
HPC = 8          # heads per core
DC = HPC * D     # 512 sharded feature cols per core
EC = E // 128    # 8 e-chunks
TT = S // 128    # 16 token tiles
QCH = S // 512   # 4 query chunks
NB = S // 128    # 16 key blocks

BF16 = ml_dtypes.bfloat16

_CACHE = {}


def _build():
    import concourse.tile as tile
    from concourse import bacc, mybir

    F32 = mybir.dt.float32
    BF = mybir.dt.bfloat16
    AF = mybir.ActivationFunctionType
    ALU = mybir.AluOpType

    nc = bacc.Bacc("TRN2", target_bir_lowering=False, debug=False, num_devices=8)

    xT_d = nc.dram_tensor("xT", [EC, 128, S], BF, kind="ExternalInput")
    wq_d = nc.dram_tensor("wq", [EC, 128, DC], BF, kind="ExternalInput")
    wk_d = nc.dram_tensor("wk", [EC, 128, DC], BF, kind="ExternalInput")
    wv_d = nc.dram_tensor("wv", [EC, 128, DC], BF, kind="ExternalInput")
    wo_d = nc.dram_tensor("wo", [DC // 128, 128, E], BF, kind="ExternalInput")
    bq_d = nc.dram_tensor("bq", [128, 4], F32, kind="ExternalInput")
    bk_d = nc.dram_tensor("bk", [128, 4], F32, kind="ExternalInput")
    bvr_d = nc.dram_tensor("bvr", [128, HPC, 64], F32, kind="ExternalInput")
    mask_d = nc.dram_tensor("mask", [128, 128], BF, kind="ExternalInput")
    out_d = nc.dram_tensor("out", [TT, 128, E], BF, kind="ExternalOutput")

    with tile.TileContext(nc) as tc:
        with tc.tile_pool(name="const", bufs=1) as cp, \
             tc.tile_pool(name="expp", bufs=1) as expp, \
             tc.tile_pool(name="work", bufs=2) as wp, \
             tc.tile_pool(name="aou", bufs=2) as aop, \
             tc.tile_pool(name="rcp", bufs=2) as rp, \
             tc.tile_pool(name="rcp1", bufs=1) as rp1, \
             tc.tile_pool(name="ps_s", bufs=2, space="PSUM") as ps_s, \
             tc.tile_pool(name="ps_av", bufs=3, space="PSUM") as ps_av, \
             tc.tile_pool(name="ps_w", bufs=1, space="PSUM") as ps_w:

            # ---- persistent SBUF tensors + input DMAs ----
            xT = [cp.tile([128, S], BF, tag=f"xT{k}", name=f"xT{k}") for k in range(EC)]
            wq = [cp.tile([128, DC], BF, tag=f"wq{k}", name=f"wq{k}") for k in range(EC)]
            wk = [cp.tile([128, DC], BF, tag=f"wk{k}", name=f"wk{k}") for k in range(EC)]
            wv = [cp.tile([128, DC], BF, tag=f"wv{k}", name=f"wv{k}") for k in range(EC)]
            wo = [cp.tile([128, E], BF, tag=f"wo{k}", name=f"wo{k}") for k in range(DC // 128)]
            # DMA order matters for startup: xT streams on one queue, weights
            # (q, then k, then v, then o, in consumption order) on another, so
            # the first projections pace with arrivals instead of waiting for
            # the full 8 MB.
            for k in range(EC):
                nc.sync.dma_start(xT[k][:], xT_d.ap()[k])
            for k in range(EC):
                nc.gpsimd.dma_start(wq[k][:], wq_d.ap()[k])
            for k in range(EC):
                nc.gpsimd.dma_start(wk[k][:], wk_d.ap()[k])
            for k in range(EC):
                nc.gpsimd.dma_start(wv[k][:], wv_d.ap()[k])
            for k in range(DC // 128):
                nc.gpsimd.dma_start(wo[k][:], wo_d.ap()[k])
            bq = cp.tile([128, 4], F32, tag="bq", name="bq")
            bk = cp.tile([128, 4], F32, tag="bk", name="bk")
            bvr = cp.tile([128, HPC, 64], F32, tag="bvr", name="bvr")
            mask = cp.tile([128, 128], BF, tag="mask", name="mask")
            nc.sync.dma_start(bq[:], bq_d.ap())
            nc.sync.dma_start(bk[:], bk_d.ap())
            nc.sync.dma_start(bvr[:], bvr_d.ap())
            nc.sync.dma_start(mask[:], mask_d.ap())
            # ones row for the rank-1 recip broadcasts
            ones = cp.tile([1, 64], BF, tag="ones", name="ones")
            nc.any.memset(ones[:], 1.0)

            # HAM warm-up: a dense burst un-throttles the PE clock gate at
            # t=0, then a PE->DVE->PE chain (enforced by the single psw
            # buffer) paces ~1us-spaced matmuls across the whole input-DMA
            # phase so the gate never re-throttles before real work starts
            wmt = cp.tile([128, 512], BF, tag="wmt", name="wmt")
            wms = cp.tile([128, 512], BF, tag="wms", name="wms")
            nc.any.memset(wmt[:], 0.0)
            for w in range(28):
                psx = ps_w.tile([128, 512], F32, tag="psw", name="psw")
                nc.tensor.matmul(psx[:], wmt[:, 0:128], wmt[:],
                                 start=True, stop=True)
                if w >= 8:
                    nc.vector.tensor_copy(out=wms[:], in_=psx[:])

            QT = [cp.tile([128, S], BF, tag=f"QT{t}", name=f"QT{t}") for t in range(4)]
            KT = [cp.tile([128, S], BF, tag=f"KT{t}", name=f"KT{t}") for t in range(4)]
            V = [cp.tile([128, HPC, 66], BF, tag=f"V{s}", name=f"V{s}") for s in range(TT)]
            AOT = [cp.tile([128, S], BF, tag=f"AOT{t}", name=f"AOT{t}") for t in range(4)]

            # Projection / out-projection work is emitted as "filler"
            # interleaved into the attention stream: attention is
            # ScalarE(exp)-paced with PE spare capacity, while fillers are
            # pure dense PE work.
            filler = []

            def proj_group(w_sb, b_sb, dst, t, qc):
                def emit():
                    ps = ps_w.tile([128, 512], F32, tag="psw", name="psw")
                    for k in range(EC):
                        nc.tensor.matmul(
                            ps[:],
                            w_sb[k][:, t * 128:(t + 1) * 128],
                            xT[k][:, qc * 512:(qc + 1) * 512],
                            start=(k == 0), stop=(k == EC - 1))
                    nc.vector.tensor_scalar(
                        dst[t][:, qc * 512:(qc + 1) * 512], ps[:],
                        b_sb[:, t:t + 1], None, ALU.add)
                return emit

            def v_group(s):
                def emit():
                    ps = ps_w.tile([128, 512], F32, tag="psw", name="psw")
                    for k in range(EC):
                        nc.tensor.matmul(
                            ps[:],
                            xT[k][:, s * 128:(s + 1) * 128],
                            wv[k][:],
                            start=(k == 0), stop=(k == EC - 1))
                    nc.vector.tensor_tensor(
                        V[s][:, :, 0:64],
                        ps[:].rearrange("p (h d) -> p h d", d=64),
                        bvr[:], ALU.add)
                    nc.any.memset(V[s][:, :, 64:65], 1.0)
                return emit

            def d_group(s):
                def emit():
                    osb = wp.tile([128, E], BF, tag="osb", name="osb")
                    for n in range(2):
                        ps = ps_w.tile([128, 512], F32, tag="psw", name="psw")
                        for k in range(DC // 128):
                            nc.tensor.matmul(
                                ps[:],
                                AOT[k][:, s * 128:(s + 1) * 128],
                                wo[k][:, n * 512:(n + 1) * 512],
                                start=(k == 0), stop=(k == DC // 128 - 1))
                        nc.vector.tensor_copy(out=osb[:, n * 512:(n + 1) * 512],
                                              in_=ps[:])
                    nc.sync.dma_start(out_d.ap()[s], osb[:])
                return emit

            # up-front: pair-0 projections + first V tiles (attention prologue)
            for t in range(4):
                for qc in range(QCH):
                    if t == 0:
                        proj_group(wq, bq, QT, t, qc)()
                        proj_group(wk, bk, KT, t, qc)()
                    else:
                        filler.append(("qkt", t, proj_group(wq, bq, QT, t, qc)))
                        filler.append(("qkt", t, proj_group(wk, bk, KT, t, qc)))
            for s in range(TT):
                if s < 4:
                    v_group(s)()
                else:
                    filler.append(("v", s, v_group(s)))

            def emit_filler_until(pred_drop):
                keep = []
                for item in filler:
                    if pred_drop(item):
                        item[2]()
                    else:
                        keep.append(item)
                filler[:] = keep

            def emit_some_filler(n):
                for _ in range(min(n, len(filler))):
                    filler.pop(0)[2]()

            def norm_group(qc, rcpb, aou, hp):
                """Deferred normalization for one head pair: two rank-1
                bcast matmuls + two [64,512] multiplies. All inputs were
                finalized a phase ago -> no cross-engine stalls in the PE
                stream. Runs as a filler."""
                def emit():
                    psb = ps_w.tile([128, 512], F32, tag="psw", name="psw")
                    for i, cpos in ((0, 0), (1, 64)):
                        nc.tensor.matmul(
                            psb[cpos:cpos + 64, :], ones[:],
                            rcpb[0:1, i, :], start=True, stop=True,
                            tile_position=(0, cpos))
                    nc.vector.tensor_tensor(
                        AOT[hp][:, qc * 512:(qc + 1) * 512],
                        aou[:], psb[:], ALU.mult)
                return emit

            # ---- attention, head-pair interleaved, qc-outer ----
            # qc=3 (the longest phase) runs first so it absorbs the
            # projection fillers; V tiles are forced in per-round just before
            # the key block that consumes them.
            for qc in (3, 0, 1, 2):
                nkb = 4 * qc + 4
                norm_args = {}
                for hp in range(4):
                    emit_filler_until(lambda it: it[0] == "qkt" and it[1] <= hp)
                    hA, hB = 2 * hp, 2 * hp + 1
                    pav = {h: ps_av.tile([128, 512], F32, tag="pav", name="pav")
                           for h in (hA, hB)}
                    expT = expp.tile([128, 2, NB, 512], BF, tag="expT",
                                     name="expT")
                    aou = aop.tile([128, 512], BF, tag=f"aou{hp}",
                                   name=f"aou{hp}")
                    # software pipeline in 2-kb superrounds: all 4 score MMs
                    # (K=64, dual-issuing) then all 4 AV MMs (K=128) of the
                    # superround 2 kb behind — halves the K64<->K128 LDW-
                    # serialization boundaries and keeps the PE off ScalarE's
                    # critical path (the exp an AV needs finished a whole
                    # superround ago)
                    LAG = 2

                    def av_off(kb):
                        dj = kb - 4 * qc
                        return 128 * dj if dj > 0 else 0

                    for st in range(0, nkb + LAG, 2):
                        for kb in (st, st + 1):
                            if kb >= nkb:
                                continue
                            dj = kb - 4 * qc
                            off = av_off(kb)
                            pss = ps_s.tile([128, 2, 512], F32, tag="pss",
                                            name="pss")
                            for i, r in ((0, 0), (1, 64)):
                                nc.tensor.matmul(
                                    pss[:, i, off:512],
                                    KT[hp][r:r + 64, kb * 128:(kb + 1) * 128],
                                    QT[hp][r:r + 64,
                                           qc * 512 + off:(qc + 1) * 512],
                                    start=True, stop=True)
                            nc.scalar.activation(
                                expT[:, :, kb, off:512], pss[:, :, off:512],
                                AF.Exp, scale=0.125)
                            if dj >= 0:
                                for i in (0, 1):
                                    nc.gpsimd.tensor_tensor(
                                        expT[:, i, kb, off:off + 128],
                                        expT[:, i, kb, off:off + 128],
                                        mask[:], ALU.mult)
                        for akb in (st - LAG, st - LAG + 1):
                            if not (0 <= akb < nkb):
                                continue
                            emit_filler_until(
                                lambda it: it[0] == "v" and it[1] <= akb)
                            off = av_off(akb)
                            for i, h in ((0, hA), (1, hB)):
                                nc.tensor.matmul(
                                    pav[h][0:65, off:512],
                                    V[akb][:, h, 0:65],
                                    expT[:, i, akb, off:512],
                                    start=(akb == 0), stop=(akb == nkb - 1))
                        # extra filler during pipeline refill at pair start
                        emit_some_filler(3 if st < LAG else 2)
                    # pair end: cheap copies + 1-partition recip at base 0
                    # (reciprocal_approx_fast only works at partition base 0
                    # with SBUF input); everything else deferred
                    dsb = rp1.tile([1, 2, 512], F32, tag="dsb", name="dsb")
                    rcpf = rp1.tile([1, 2, 512], F32, tag="rcpf", name="rcpf")
                    rcpb = rp.tile([1, 2, 512], BF, tag=f"rcpb{hp}",
                                   name=f"rcpb{hp}")
                    for i, (h, r) in ((0, (hA, 0)), (1, (hB, 64))):
                        nc.vector.tensor_copy(out=aou[r:r + 64, :],
                                              in_=pav[h][0:64, :])
                        nc.vector.tensor_copy(out=dsb[0:1, i, :],
                                              in_=pav[h][64:65, :])
                    nc.vector.reciprocal_approx_fast(out=rcpf[:], in_=dsb[:])
                    nc.vector.tensor_copy(out=rcpb[:], in_=rcpf[:])
                    norm_args[hp] = (rcpb, aou)
                # normalization + out-projection for this qc become fillers
                # for the next phase (norms first so the d_groups' AOT
                # dependencies resolve in FIFO order)
                for hp in range(4):
                    rcpb, aou = norm_args[hp]
                    filler.append(("n", hp, norm_group(qc, rcpb, aou, hp)))
                for s in range(qc * 4, qc * 4 + 4):
                    filler.append(("d", s, d_group(s)))
            emit_filler_until(lambda it: True)

    nc.compile()
    return nc


def _get_nc():
    if "nc" not in _CACHE:
        _CACHE["nc"] = _build()
    return _CACHE["nc"]


def _shard_inputs(x, Wq, bq, Wk, bk, Wv, bv, Wo):
    """Build the 8 per-core input maps (host-side shard/cast/transpose)."""
    x = np.asarray(x, np.float32)
    mask = np.triu(np.ones((128, 128), np.float32)).astype(BF16)  # [k, q] q>=k
    in_maps = []
    for c in range(8):
        b, hg = divmod(c, 2)
        dc = slice(hg * DC, (hg + 1) * DC)
        xT = np.ascontiguousarray(x[b].T).astype(BF16).reshape(EC, 128, S)
        wq_c = np.ascontiguousarray(Wq[:, dc]).astype(BF16).reshape(EC, 128, DC)
        wk_c = np.ascontiguousarray(Wk[:, dc]).astype(BF16).reshape(EC, 128, DC)
        wv_c = np.ascontiguousarray(Wv[:, dc]).astype(BF16).reshape(EC, 128, DC)
        wo_c = np.ascontiguousarray(Wo[dc, :]).astype(BF16).reshape(DC // 128, 128, E)
        bq_c = np.ascontiguousarray(np.asarray(bq[dc], np.float32).reshape(4, 128).T)
        bk_c = np.ascontiguousarray(np.asarray(bk[dc], np.float32).reshape(4, 128).T)
        bvr_c = np.ascontiguousarray(np.broadcast_to(
            np.asarray(bv[dc], np.float32).reshape(1, HPC, 64),
            (128, HPC, 64)))
        in_maps.append({
            "xT": xT, "wq": wq_c, "wk": wk_c, "wv": wv_c, "wo": wo_c,
            "bq": bq_c, "bk": bk_c, "bvr": bvr_c, "mask": mask,
        })
    return in_maps


def kernel(x, Wq, bq, Wk, bk, Wv, bv, Wo, bo):
    from concourse.bass_utils import run_bass_kernel_spmd

    nc = _get_nc()
    in_maps = _shard_inputs(x, Wq, bq, Wk, bk, Wv, bv, Wo)
    res = run_bass_kernel_spmd(nc, in_maps, core_ids=list(range(8)))
    bo = np.asarray(bo, np.float32)
    out = np.empty((B, S, E), np.float32)
    for b in range(B):
        p0 = res.results[2 * b]["out"].astype(np.float32).reshape(S, E)
        p1 = res.results[2 * b + 1]["out"].astype(np.float32).reshape(S, E)
        out[b] = p0 + p1 + bo
    return out


# revision 21
# speedup vs baseline: 1.1518x; 1.1518x over previous
"""Multi-head self-attention (B=4, S=2048, E=1024, H=16, causal) on 8 NeuronCores.

Sharding (Megatron-style, per hint): data-parallel over B (4) x tensor-parallel
over heads (2 groups of 8 heads). Core c handles batch c//2 with head-group
c%2: Wq/Wk/Wv sharded column-wise, Wo row-wise. Each core emits a partial
out-projection [S, E]; the host sums each pair of partials (the "all-reduce")
and adds bo.

Per-core kernel (all matmuls bf16, fp32 accumulation):
  - host supplies x[b].T so Q^T,K^T ([d,s]) and V ([s,d]) come straight off
    the projections with no on-chip transposes
  - scores computed transposed (S^T = K Q^T, [keys, queries]) with causal
    block-skipping; both heads of a pair run as concurrent row-tiled matmuls
    into one 2-bank PSUM tile; one exp per round covers both heads
  - softmax denominator comes free from a ones-column appended to V in the
    attn @ V matmul; V-bias is folded into V itself (sum attn = denom, so
    (sum attn*(v+bv))/denom = AV/denom + bv)
  - normalization is pulled OFF the critical path: per pair only two cheap
    PSUM->SBUF copies (unnormalized AV + 1/denom via the 1-instr
    reciprocal_approx_fast); the rank-1 broadcast matmul and the multiply
    are deferred a full phase, so the PE never stalls on VectorE
  - projection bias adds moved from ScalarE to VectorE (ScalarE = exp only)
  - out-projection partials stored/DMA'd bf16 (host reduces in fp32)
"""

import numpy as np
import ml_dtypes

B, S, E, H, D = 4, 2048, 1024, 16, 64
HPC = 8          # heads per core
DC = HPC * D     # 512 sharded feature cols per core
EC = E // 128    # 8 e-chunks
TT = S // 128    # 16 token tiles
QCH = S // 512   # 4 query chunks
NB = S // 128    # 16 key blocks

BF16 = ml_dtypes.bfloat16

_CACHE = {}


def _build():
    import concourse.tile as tile
    from concourse import bacc, mybir

    F32 = mybir.dt.float32
    BF = mybir.dt.bfloat16
    AF = mybir.ActivationFunctionType
    ALU = mybir.AluOpType

    nc = bacc.Bacc("TRN2", target_bir_lowering=False, debug=False, num_devices=8)

    xT_d = nc.dram_tensor("xT", [EC, 128, S], BF, kind="ExternalInput")
    wq_d = nc.dram_tensor("wq", [EC, 128, DC], BF, kind="ExternalInput")
    wk_d = nc.dram_tensor("wk", [EC, 128, DC], BF, kind="ExternalInput")
    wv_d = nc.dram_tensor("wv", [EC, 128, DC], BF, kind="ExternalInput")
    wo_d = nc.dram_tensor("wo", [DC // 128, 128, E], BF, kind="ExternalInput")
    bq_d = nc.dram_tensor("bq", [128, 4], F32, kind="ExternalInput")
    bk_d = nc.dram_tensor("bk", [128, 4], F32, kind="ExternalInput")
    bvr_d = nc.dram_tensor("bvr", [128, HPC, 64], F32, kind="ExternalInput")
    mask_d = nc.dram_tensor("mask", [128, 128], BF, kind="ExternalInput")
    out_d = nc.dram_tensor("out", [TT, 128, E], BF, kind="ExternalOutput")

    with tile.TileContext(nc) as tc:
        with tc.tile_pool(name="const", bufs=1) as cp, \
             tc.tile_pool(name="expp", bufs=1) as expp, \
             tc.tile_pool(name="work", bufs=2) as wp, \
             tc.tile_pool(name="aou", bufs=2) as aop, \
             tc.tile_pool(name="rcp", bufs=2) as rp, \
             tc.tile_pool(name="rcp1", bufs=1) as rp1, \
             tc.tile_pool(name="ps_s", bufs=2, space="PSUM") as ps_s, \
             tc.tile_pool(name="ps_av", bufs=2, space="PSUM") as ps_av, \
             tc.tile_pool(name="ps_w", bufs=2, space="PSUM") as ps_w:

            # ---- persistent SBUF tensors + input DMAs ----
            xT = [cp.tile([128, S], BF, tag=f"xT{k}", name=f"xT{k}") for k in range(EC)]
            wq = [cp.tile([128, DC], BF, tag=f"wq{k}", name=f"wq{k}") for k in range(EC)]
            wk = [cp.tile([128, DC], BF, tag=f"wk{k}", name=f"wk{k}") for k in range(EC)]
            wv = [cp.tile([128, DC], BF, tag=f"wv{k}", name=f"wv{k}") for k in range(EC)]
            wo = [cp.tile([128, E], BF, tag=f"wo{k}", name=f"wo{k}") for k in range(DC // 128)]
            # DMA order matters for startup: xT streams on one queue, weights
            # (q, then k, then v, then o, in consumption order) on another, so
            # the first projections pace with arrivals instead of waiting for
            # the full 8 MB.
            for k in range(EC):
                nc.sync.dma_start(xT[k][:], xT_d.ap()[k])
            for k in range(EC):
                nc.gpsimd.dma_start(wq[k][:], wq_d.ap()[k])
            for k in range(EC):
                nc.gpsimd.dma_start(wk[k][:], wk_d.ap()[k])
            for k in range(EC):
                nc.gpsimd.dma_start(wv[k][:], wv_d.ap()[k])
            for k in range(DC // 128):
                nc.gpsimd.dma_start(wo[k][:], wo_d.ap()[k])
            bq = cp.tile([128, 4], F32, tag="bq", name="bq")
            bk = cp.tile([128, 4], F32, tag="bk", name="bk")
            bvr = cp.tile([128, HPC, 64], F32, tag="bvr", name="bvr")
            mask = cp.tile([128, 128], BF, tag="mask", name="mask")
            nc.sync.dma_start(bq[:], bq_d.ap())
            nc.sync.dma_start(bk[:], bk_d.ap())
            nc.sync.dma_start(bvr[:], bvr_d.ap())
            nc.sync.dma_start(mask[:], mask_d.ap())
            # ones row for the rank-1 recip broadcasts
            ones = cp.tile([1, 64], BF, tag="ones", name="ones")
            nc.any.memset(ones[:], 1.0)

            # HAM warm-up: a dense burst un-throttles the PE clock gate at
            # t=0, then a PE->DVE->PE chain (enforced by the single psw
            # buffer) paces ~1us-spaced matmuls across the whole input-DMA
            # phase so the gate never re-throttles before real work starts
            wmt = cp.tile([128, 512], BF, tag="wmt", name="wmt")
            wms = cp.tile([128, 512], BF, tag="wms", name="wms")
            nc.any.memset(wmt[:], 0.0)
            for w in range(28):
                psx = ps_w.tile([128, 512], F32, tag="psw", name="psw")
                nc.tensor.matmul(psx[:], wmt[:, 0:128], wmt[:],
                                 start=True, stop=True)
                if w >= 8:
                    nc.vector.tensor_copy(out=wms[:], in_=psx[:])

            QT = [cp.tile([128, S], BF, tag=f"QT{t}", name=f"QT{t}") for t in range(4)]
            KT = [cp.tile([128, S], BF, tag=f"KT{t}", name=f"KT{t}") for t in range(4)]
            V = [cp.tile([128, HPC, 66], BF, tag=f"V{s}", name=f"V{s}") for s in range(TT)]
            AOT = [cp.tile([128, S], BF, tag=f"AOT{t}", name=f"AOT{t}") for t in range(4)]

            # Projection / out-projection work is emitted as "filler"
            # interleaved into the attention stream: attention is
            # ScalarE(exp)-paced with PE spare capacity, while fillers are
            # pure dense PE work.
            filler = []

            def proj_group(w_sb, b_sb, dst, t, qc):
                def emit():
                    ps = ps_w.tile([128, 512], F32, tag="psw", name="psw")
                    for k in range(EC):
                        nc.tensor.matmul(
                            ps[:],
                            w_sb[k][:, t * 128:(t + 1) * 128],
                            xT[k][:, qc * 512:(qc + 1) * 512],
                            start=(k == 0), stop=(k == EC - 1))
                    nc.vector.tensor_scalar(
                        dst[t][:, qc * 512:(qc + 1) * 512], ps[:],
                        b_sb[:, t:t + 1], None, ALU.add)
                return emit

            def v_group(s):
                def emit():
                    ps = ps_w.tile([128, 512], F32, tag="psw", name="psw")
                    for k in range(EC):
                        nc.tensor.matmul(
                            ps[:],
                            xT[k][:, s * 128:(s + 1) * 128],
                            wv[k][:],
                            start=(k == 0), stop=(k == EC - 1))
                    nc.vector.tensor_tensor(
                        V[s][:, :, 0:64],
                        ps[:].rearrange("p (h d) -> p h d", d=64),
                        bvr[:], ALU.add)
                    nc.any.memset(V[s][:, :, 64:65], 1.0)
                return emit

            def d_group(s):
                def emit():
                    osb = wp.tile([128, E], BF, tag="osb", name="osb")
                    for n in range(2):
                        ps = ps_w.tile([128, 512], F32, tag="psw", name="psw")
                        for k in range(DC // 128):
                            nc.tensor.matmul(
                                ps[:],
                                AOT[k][:, s * 128:(s + 1) * 128],
                                wo[k][:, n * 512:(n + 1) * 512],
                                start=(k == 0), stop=(k == DC // 128 - 1))
                        nc.vector.tensor_copy(out=osb[:, n * 512:(n + 1) * 512],
                                              in_=ps[:])
                    nc.sync.dma_start(out_d.ap()[s], osb[:])
                return emit

            # up-front: pair-0 projections + first V tiles (attention prologue)
            for t in range(4):
                for qc in range(QCH):
                    if t == 0:
                        proj_group(wq, bq, QT, t, qc)()
                        proj_group(wk, bk, KT, t, qc)()
                    else:
                        filler.append(("qkt", t, proj_group(wq, bq, QT, t, qc)))
                        filler.append(("qkt", t, proj_group(wk, bk, KT, t, qc)))
            for s in range(TT):
                if s < 4:
                    v_group(s)()
                else:
                    filler.append(("v", s, v_group(s)))

            def emit_filler_until(pred_drop):
                keep = []
                for item in filler:
                    if pred_drop(item):
                        item[2]()
                    else:
                        keep.append(item)
                filler[:] = keep

            def emit_some_filler(n):
                for _ in range(min(n, len(filler))):
                    filler.pop(0)[2]()

            def norm_group(qc, rcpb, aou, hp):
                """Deferred normalization for one head pair: two rank-1
                bcast matmuls + two [64,512] multiplies. All inputs were
                finalized a phase ago -> no cross-engine stalls in the PE
                stream. Runs as a filler."""
                def emit():
                    psb = ps_w.tile([128, 512], F32, tag="psw", name="psw")
                    for i, cpos in ((0, 0), (1, 64)):
                        nc.tensor.matmul(
                            psb[cpos:cpos + 64, :], ones[:],
                            rcpb[0:1, i, :], start=True, stop=True,
                            tile_position=(0, cpos))
                    nc.vector.tensor_tensor(
                        AOT[hp][:, qc * 512:(qc + 1) * 512],
                        aou[:], psb[:], ALU.mult)
                return emit

            # ---- attention, head-pair interleaved, qc-outer ----
            # qc=3 (the longest phase) runs first so it absorbs the
            # projection fillers; V tiles are forced in per-round just before
            # the key block that consumes them.
            for qc in (3, 0, 1, 2):
                nkb = 4 * qc + 4
                norm_args = {}
                for hp in range(4):
                    emit_filler_until(lambda it: it[0] == "qkt" and it[1] <= hp)
                    hA, hB = 2 * hp, 2 * hp + 1
                    pav = {h: ps_av.tile([128, 512], F32, tag="pav", name="pav")
                           for h in (hA, hB)}
                    expT = expp.tile([128, 2, NB, 512], BF, tag="expT",
                                     name="expT")
                    aou = aop.tile([128, 512], BF, tag=f"aou{hp}",
                                   name=f"aou{hp}")
                    # software pipeline in 2-kb superrounds: all 4 score MMs
                    # (K=64, dual-issuing) then all 4 AV MMs (K=128) of the
                    # superround 2 kb behind — halves the K64<->K128 LDW-
                    # serialization boundaries and keeps the PE off ScalarE's
                    # critical path (the exp an AV needs finished a whole
                    # superround ago)
                    LAG = 2

                    def av_off(kb):
                        dj = kb - 4 * qc
                        return 128 * dj if dj > 0 else 0

                    for st in range(0, nkb + LAG, 2):
                        for kb in (st, st + 1):
                            if kb >= nkb:
                                continue
                            dj = kb - 4 * qc
                            off = av_off(kb)
                            pss = ps_s.tile([128, 2, 512], F32, tag="pss",
                                            name="pss")
                            for i, r in ((0, 0), (1, 64)):
                                nc.tensor.matmul(
                                    pss[:, i, off:512],
                                    KT[hp][r:r + 64, kb * 128:(kb + 1) * 128],
                                    QT[hp][r:r + 64,
                                           qc * 512 + off:(qc + 1) * 512],
                                    start=True, stop=True)
                            nc.scalar.activation(
                                expT[:, :, kb, off:512], pss[:, :, off:512],
                                AF.Exp, scale=0.125)
                            if dj >= 0:
                                for i in (0, 1):
                                    nc.gpsimd.tensor_tensor(
                                        expT[:, i, kb, off:off + 128],
                                        expT[:, i, kb, off:off + 128],
                                        mask[:], ALU.mult)
                        for akb in (st - LAG, st - LAG + 1):
                            if not (0 <= akb < nkb):
                                continue
                            emit_filler_until(
                                lambda it: it[0] == "v" and it[1] <= akb)
                            off = av_off(akb)
                            for i, h in ((0, hA), (1, hB)):
                                nc.tensor.matmul(
                                    pav[h][0:65, off:512],
                                    V[akb][:, h, 0:65],
                                    expT[:, i, akb, off:512],
                                    start=(akb == 0), stop=(akb == nkb - 1))
                        # extra filler during pipeline refill at pair start
                        emit_some_filler(3 if st < LAG else 2)
                    # pair end: cheap copies + 1-partition recip at base 0
                    # (reciprocal_approx_fast only works at partition base 0
                    # with SBUF input); everything else deferred
                    dsb = rp1.tile([1, 2, 512], F32, tag="dsb", name="dsb")
                    rcpf = rp1.tile([1, 2, 512], F32, tag="rcpf", name="rcpf")
                    rcpb = rp.tile([1, 2, 512], BF, tag=f"rcpb{hp}",
                                   name=f"rcpb{hp}")
                    for i, (h, r) in ((0, (hA, 0)), (1, (hB, 64))):
                        nc.vector.tensor_copy(out=aou[r:r + 64, :],
                                              in_=pav[h][0:64, :])
                        nc.vector.tensor_copy(out=dsb[0:1, i, :],
                                              in_=pav[h][64:65, :])
                    nc.vector.reciprocal_approx_fast(out=rcpf[:], in_=dsb[:])
                    nc.vector.tensor_copy(out=rcpb[:], in_=rcpf[:])
                    norm_args[hp] = (rcpb, aou)
                # normalization + out-projection for this qc become fillers
                # for the next phase (norms first so the d_groups' AOT
                # dependencies resolve in FIFO order)
                for hp in range(4):
                    rcpb, aou = norm_args[hp]
                    filler.append(("n", hp, norm_group(qc, rcpb, aou, hp)))
                for s in range(qc * 4, qc * 4 + 4):
                    filler.append(("d", s, d_group(s)))
            emit_filler_until(lambda it: True)

    nc.compile()
    return nc


def _get_nc():
    if "nc" not in _CACHE:
        _CACHE["nc"] = _build()
    return _CACHE["nc"]


def _shard_inputs(x, Wq, bq, Wk, bk, Wv, bv, Wo):
    """Build the 8 per-core input maps (host-side shard/cast/transpose)."""
    x = np.asarray(x, np.float32)
    mask = np.triu(np.ones((128, 128), np.float32)).astype(BF16)  # [k, q] q>=k
    in_maps = []
    for c in range(8):
        b, hg = divmod(c, 2)
        dc = slice(hg * DC, (hg + 1) * DC)
        xT = np.ascontiguousarray(x[b].T).astype(BF16).reshape(EC, 128, S)
        wq_c = np.ascontiguousarray(Wq[:, dc]).astype(BF16).reshape(EC, 128, DC)
        wk_c = np.ascontiguousarray(Wk[:, dc]).astype(BF16).reshape(EC, 128, DC)
        wv_c = np.ascontiguousarray(Wv[:, dc]).astype(BF16).reshape(EC, 128, DC)
        wo_c = np.ascontiguousarray(Wo[dc, :]).astype(BF16).reshape(DC // 128, 128, E)
        bq_c = np.ascontiguousarray(np.asarray(bq[dc], np.float32).reshape(4, 128).T)
        bk_c = np.ascontiguousarray(np.asarray(bk[dc], np.float32).reshape(4, 128).T)
        bvr_c = np.ascontiguousarray(np.broadcast_to(
            np.asarray(bv[dc], np.float32).reshape(1, HPC, 64),
            (128, HPC, 64)))
        in_maps.append({
            "xT": xT, "wq": wq_c, "wk": wk_c, "wv": wv_c, "wo": wo_c,
            "bq": bq_c, "bk": bk_c, "bvr": bvr_c, "mask": mask,
        })
    return in_maps


def kernel(x, Wq, bq, Wk, bk, Wv, bv, Wo, bo):
    from concourse.bass_utils import run_bass_kernel_spmd

    nc = _get_nc()
    in_maps = _shard_inputs(x, Wq, bq, Wk, bk, Wv, bv, Wo)
    res = run_bass_kernel_spmd(nc, in_maps, core_ids=list(range(8)))
    bo = np.asarray(bo, np.float32)
    out = np.empty((B, S, E), np.float32)
    for b in range(B):
        p0 = res.results[2 * b]["out"].astype(np.float32).reshape(S, E)
        p1 = res.results[2 * b + 1]["out"].astype(np.float32).reshape(S, E)
        out[b] = p0 + p1 + bo
    return out
